# revision 9
# baseline (speedup 1.0000x reference)
"""Trainium2 Bass kernel for the ExemplarHead classification problem.

Math: per (task, way), with R the 5x1024 class reps (support+noise),
H = I - (1/5)11^T, G = H R R^T H, the SVD projection head reduces to

    C   = W R,  W = I - lam*(lam I + G)^{-1} H          (block-diag 100x100)
    logits[q, (w,s)] = (2 q.C_(w,s) - ||q||^2 - ||C_(w,s)||^2) / d

(lam I + G) has kappa <= 1.2, inverted with one scaled Newton step
(Y1 = 2I - aK, one quadratic refinement). All 20 (task,way) blocks per
core are one masked block-diagonal 100x100 problem.

All heavy matmuls run in bf16 (1 cyc/row on the PE vs 4 for fp32, and
half the LDWEIGHTS+MATMUL instruction count); accumulation stays fp32
in PSUM. q^T arrives pre-transposed from DRAM (host packs it), which
removes all PE transposes. ||q||^2 is computed per task on DVE/ACT/
GPSIMD in parallel with the PE pipeline and folded into the epilogue
as a per-partition scalar; -0.5||C||^2 is folded in as a K=1 rank-1
matmul into the accumulating QC PSUM group.

Sharding: data-parallel over the 32 tasks -> 4 tasks per NeuronCore x 8.
"""

import numpy as np
import ml_dtypes

import concourse.bass as bass
import concourse.mybir as mybir
import concourse.tile as tile
from concourse import bacc
from concourse.bass_utils import run_bass_kernel_spmd

F32 = mybir.dt.float32
BF16 = mybir.dt.bfloat16
AF = mybir.ActivationFunctionType
ALU = mybir.AluOpType

LAM = 100000.0
GMAX_BOUND = 40000.0            # safe bound on ||G|| (observed max ~2.2e4)
ALPHA = 2.0 / (2.0 * LAM + GMAX_BOUND)

N_CORES = 8
T_FULL, NQ, D = 32, 75, 1024
NW, NS = 5, 5
TPC = T_FULL // N_CORES          # tasks per core = 4
NR = TPC * NW * NS               # R rows per core = 100
NCH = D // 128                   # 8 contraction chunks
NJ = NW * NS                     # 25 (way,shot) pairs per task

# packed bf16 constant tile [128, 476]:
#   cols   0:100  cH    = H block-diagonal (I - 1/5 11^T per 5-block)
#   cols 100:200  amask = ALPHA * blockmask
#   cols 200:300  cI    = identity (diag consts derived on DVE via STT)
#   cols 300:400  alH   = ALPHA*LAM * H block-diagonal
#   col  400      negh  = -0.5 column (all 128 partitions)
#   row0 401:476  ones75
CPC = 476


def _host_consts():
    H5 = np.eye(NS) - np.ones((NS, NS)) / NS
    H_bd = np.kron(np.eye(TPC * NW), H5).astype(np.float32)
    blockmask = np.kron(np.eye(TPC * NW), np.ones((NS, NS))).astype(np.float32)
    eye = np.eye(NR, dtype=np.float32)
    cP = np.zeros((128, CPC), dtype=np.float32)
    cP[0:NR, 0:NR] = H_bd
    cP[0:NR, NR:2 * NR] = ALPHA * blockmask
    cP[0:NR, 2 * NR:3 * NR] = eye
    cP[0:NR, 3 * NR:4 * NR] = ALPHA * LAM * H_bd
    cP[:, 400] = -0.5
    cP[0, 401:401 + NQ] = 1.0
    return cP.astype(ml_dtypes.bfloat16)


def build_nc():
    nc = bacc.Bacc("TRN2")

    r_d = nc.declare_dram_parameter("r", [NR, D], BF16, isOutput=False)
    qt_d = nc.declare_dram_parameter("qt", [128, TPC * NCH * NQ], BF16,
                                     isOutput=False)
    qn_d = nc.declare_dram_parameter("qn", [NQ, TPC * D], BF16, isOutput=False)
    cP_d = nc.declare_dram_parameter("cP", [128, CPC], BF16, isOutput=False)
    out_d = nc.declare_dram_parameter("out", [NQ, TPC * NJ], F32, isOutput=True)

    with tile.TileContext(nc) as tc:
        with (
            tc.tile_pool(name="consts", bufs=1) as consts,
            tc.tile_pool(name="sb", bufs=1) as sb,
            tc.tile_pool(name="scr", bufs=2) as scr,
            tc.tile_pool(name="pipe", bufs=2, space="PSUM") as pipe,
            tc.tile_pool(name="gp", bufs=1, space="PSUM") as gp,
            tc.tile_pool(name="cnp", bufs=1, space="PSUM") as cnp,
            tc.tile_pool(name="qcp", bufs=4, space="PSUM") as qcp,
        ):
            # ---- input DMAs; r first (heads the critical path) ----
            r_sb = sb.tile([NR, D], BF16)
            nc.gpsimd.dma_start(out=r_sb, in_=r_d[:])
            cP = consts.tile([128, CPC], BF16)
            nc.gpsimd.dma_start(out=cP, in_=cP_d[:])
            qn_sb = sb.tile([NQ, TPC * D], BF16)
            nc.sync.dma_start(out=qn_sb, in_=qn_d[:])
            # q^T on the second HWDGE ring (ACT) so it streams in parallel
            qt_sb = sb.tile([128, TPC * NCH * NQ], BF16)
            nc.sync.dma_start(out=qt_sb, in_=qt_d[:])

            c_H = cP[0:NR, 0:NR]
            c_amask = cP[0:NR, NR:2 * NR]
            c_I = cP[0:NR, 2 * NR:3 * NR]
            c_alH = cP[0:NR, 3 * NR:4 * NR]
            negh = cP[:, 400:401]
            ones75 = cP[0:1, 401:401 + NQ]

            # warm the ACT spline tables (Square/Copy) before first real use
            warm = sb.tile([1, 1], BF16)
            nc.scalar.activation(warm, cP[0:1, 0:1], AF.Square)

            # ---- RcT = (H R)^T by chunks: psum[128,100] = R_chunk^T @ H ----
            rct_sb = sb.tile([128, NCH * NR], BF16)
            for p in range(2):
                rct_ps = pipe.tile([128, 4 * NR], F32, space="PSUM", tag="pp")
                for kk in range(4):
                    k = 4 * p + kk
                    nc.tensor.matmul(rct_ps[:, kk * NR:(kk + 1) * NR],
                                     lhsT=r_sb[:, k * 128:(k + 1) * 128],
                                     rhs=c_H, start=True, stop=True)
                nc.scalar.copy(rct_sb[:, p * 4 * NR:(p + 1) * 4 * NR], rct_ps)

            # ---- G = sum_k RcT_k^T RcT_k  (= H R R^T H), fp32 in PSUM ----
            g_ps = gp.tile([NR, NR], F32, space="PSUM")
            for k in range(NCH):
                rct_k = rct_sb[:, k * NR:(k + 1) * NR]
                nc.tensor.matmul(g_ps, lhsT=rct_k, rhs=rct_k,
                                 start=(k == 0), stop=(k == NCH - 1))

            # diagonal consts derived once from identity (saves DMA bytes)
            alI_sb = sb.tile([NR, NR], BF16)
            nc.vector.tensor_scalar_mul(alI_sb, c_I, ALPHA * LAM)
            y1b_sb = sb.tile([NR, NR], BF16)
            nc.vector.tensor_scalar_mul(y1b_sb, c_I, 2.0 - ALPHA * LAM)
            twoI_sb = sb.tile([NR, NR], BF16)
            nc.vector.tensor_scalar_mul(twoI_sb, c_I, 2.0)

            # ---- one Newton step for Y ~ (alpha K)^-1, all bf16 ----
            gm_sb = sb.tile([NR, NR], BF16)
            nc.vector.tensor_mul(gm_sb, g_ps, c_amask)       # gm = a*(mask o G)
            ka_sb = sb.tile([NR, NR], BF16)
            nc.vector.tensor_add(ka_sb, alI_sb, gm_sb)       # Ka = a*lam*I + gm
            y1_sb = sb.tile([NR, NR], BF16)
            nc.vector.tensor_sub(y1_sb, y1b_sb, gm_sb)       # Y1 = (2-a*lam)I - gm
            p_ps = pipe.tile([NR, NR], F32, space="PSUM", tag="pp")
            nc.tensor.matmul(p_ps, lhsT=ka_sb, rhs=y1_sb, start=True, stop=True)
            t1_ps = pipe.tile([NR, NR], F32, space="PSUM", tag="pp")
            nc.tensor.matmul(t1_ps, lhsT=y1_sb, rhs=c_alH, start=True, stop=True)
            t1_sb = sb.tile([NR, NR], BF16)
            nc.scalar.copy(t1_sb, t1_ps)                      # T1 = Y1*alH
            qq_sb = sb.tile([NR, NR], BF16)
            nc.vector.tensor_sub(qq_sb, twoI_sb, p_ps)        # Q = 2I - Ka Y1
            w2_ps = pipe.tile([NR, NR], F32, space="PSUM", tag="pp")
            nc.tensor.matmul(w2_ps, lhsT=t1_sb, rhs=qq_sb, start=True, stop=True)
            wt_sb = sb.tile([NR, NR], BF16)
            nc.vector.tensor_sub(wt_sb, c_I, w2_ps)           # W^T = I - alH Y1 Q

            # ---- C^T chunks = R_chunk^T @ W^T ; squares for ||C||^2 ----
            ct_sb = sb.tile([128, NCH * NR], BF16)
            csq_sb = sb.tile([128, NCH * NR], BF16)
            for p in range(2):
                ct_ps = pipe.tile([128, 4 * NR], F32, space="PSUM", tag="pp")
                for kk in range(4):
                    k = 4 * p + kk
                    nc.tensor.matmul(ct_ps[:, kk * NR:(kk + 1) * NR],
                                     lhsT=r_sb[:, k * 128:(k + 1) * 128],
                                     rhs=wt_sb, start=True, stop=True)
                sl = slice(p * 4 * NR, (p + 1) * 4 * NR)
                nc.vector.tensor_copy(ct_sb[:, sl], ct_ps)
                nc.scalar.activation(csq_sb[:, sl], ct_ps, AF.Square)

            # ---- cn row [1,100] = -0.5 sum_d C^T(d,j)^2, PSUM-accumulated ----
            cn_ps = cnp.tile([1, NR], F32, space="PSUM")
            for k in range(NCH):
                nc.tensor.matmul(cn_ps, lhsT=negh,
                                 rhs=csq_sb[:, k * NR:(k + 1) * NR],
                                 start=(k == 0), stop=(k == NCH - 1))
            cn_sb = sb.tile([1, NR], BF16)
            nc.scalar.copy(cn_sb, cn_ps)

            # ---- ||q||^2 per task, spread over DVE / ACT / GPSIMD ----
            qcol = [sb.tile([NQ, 1], F32, name=f"qcol{t}") for t in range(TPC)]
            for t in range(TPC):
                sq = scr.tile([NQ, D], BF16, tag="sq")
                nc.scalar.activation(sq, qn_sb[:, t * D:(t + 1) * D],
                                     AF.Square, accum_out=qcol[t])

            # ---- per task: QC accumulation + rank-1 cn + fused epilogue ----
            out_sb = sb.tile([NQ, TPC * NJ], F32)
            qnh = [sb.tile([NQ, 1], F32, name=f"qnh{t}") for t in range(TPC)]
            for t in range(TPC):
                qc_ps = qcp.tile([NQ, NJ], F32, space="PSUM", tag="qc",
                                 name=f"qc{t}")
                for k in range(NCH):
                    lhs = qt_sb[:, (t * NCH + k) * NQ:(t * NCH + k + 1) * NQ]
                    rhs = ct_sb[:, k * NR + t * NJ:k * NR + t * NJ + NJ]
                    nc.tensor.matmul(qc_ps, lhsT=lhs, rhs=rhs,
                                     start=(k == 0), stop=False)
                nc.tensor.matmul(qc_ps, lhsT=ones75,
                                 rhs=cn_sb[0:1, t * NJ:(t + 1) * NJ],
                                 start=False, stop=True)
                # logits = (qc - 0.5||q||^2) * 2/D  (cn already in qc)
                nc.vector.tensor_scalar_mul(qnh[t], qcol[t], -1.0 / D)
                tmp_t = scr.tile([NQ, NJ], F32, tag="ep")
                nc.vector.tensor_scalar_mul(tmp_t, qc_ps, 2.0 / D)
                nc.vector.tensor_scalar_add(
                    out_sb[:, t * NJ:(t + 1) * NJ], tmp_t, qnh[t][:, 0:1])
            nc.sync.dma_start(out=out_d[:], in_=out_sb)

    nc.finalize()
    return nc


_NC_CACHE = None


def _get_nc():
    global _NC_CACHE
    if _NC_CACHE is None:
        _NC_CACHE = build_nc()
    return _NC_CACHE


def make_in_maps(query, support, noise):
    query = np.asarray(query, dtype=np.float32)
    support = np.asarray(support, dtype=np.float32)
    noise = np.asarray(noise, dtype=np.float32)
    cP = _host_consts()
    in_maps = []
    for c in range(N_CORES):
        ts = slice(c * TPC, (c + 1) * TPC)
        sn = support[ts].reshape(NR, D).astype(ml_dtypes.bfloat16)
        nz = noise[:, ts].transpose(1, 0, 2, 3).reshape(NR, D) \
            .astype(ml_dtypes.bfloat16)
        r = (sn.astype(np.float32) + nz.astype(np.float32)) \
            .astype(ml_dtypes.bfloat16)
        qb = query[ts].astype(ml_dtypes.bfloat16)      # [4, 75, 1024]
        # qt[p, (t*8 + k)*75 + j] = q[t, j, k*128 + p]
        qt = np.ascontiguousarray(
            qb.transpose(2, 0, 1)                       # [1024, 4, 75]
              .reshape(NCH, 128, TPC, NQ)
              .transpose(1, 2, 0, 3)                    # [128, 4, 8, 75]
              .reshape(128, TPC * NCH * NQ))
        qn = np.ascontiguousarray(qb.transpose(1, 0, 2).reshape(NQ, TPC * D))
        in_maps.append({"r": r, "qt": qt, "cP": cP, "qn": qn})
    return in_maps


def kernel(query, support, noise, support_labels=None, n_way=None, n_shot=None,
           **_unused):
    nc = _get_nc()
    in_maps = make_in_maps(query, support, noise)
    res = run_bass_kernel_spmd(nc, in_maps, list(range(N_CORES)))
    outs = [np.asarray(r["out"]).reshape(NQ, TPC, NJ).transpose(1, 0, 2)
            for r in res.results]
    full = np.concatenate(outs, axis=0)            # (32, 75, 25)
    return full.reshape(T_FULL, NQ, NW, NS).astype(np.float32)


# revision 10
# speedup vs baseline: 1.1020x; 1.1020x over previous
"""Trainium2 Bass kernel for the ExemplarHead classification problem.

Math: per (task, way), with R the 5x1024 class reps (support+noise),
H = I - (1/5)11^T, G = H R R^T H, the SVD projection head reduces to

    C   = W R,  W = I - lam*(lam I + G)^{-1} H          (block-diag 100x100)
    logits[q, (w,s)] = (2 q.C_(w,s) - ||q||^2 - ||C_(w,s)||^2) / d

(lam I + G) has kappa <= 1.2, inverted with one scaled Newton step
(Y1 = 2I - aK, one quadratic refinement). All 20 (task,way) blocks per
core are one masked block-diagonal 100x100 problem.

All heavy matmuls run in bf16 (1 cyc/row on the PE vs 4 for fp32, and
half the LDWEIGHTS+MATMUL instruction count); accumulation stays fp32
in PSUM. q^T arrives pre-transposed from DRAM (host packs it), which
removes all PE transposes. ||q||^2 is computed from the transposed q
(element square on DVE/ACT, then a 1-column reduce matmul that hides
in the Newton-chain PE gaps); both -0.5||q||^2 and -0.5||C||^2 are
folded into the accumulating QC PSUM group as K=1 rank-1 matmuls, so
the epilogue is a single scale per task.

Sharding: data-parallel over the 32 tasks -> 4 tasks per NeuronCore x 8.
"""

import numpy as np
import ml_dtypes

import concourse.bass as bass
import concourse.mybir as mybir
import concourse.tile as tile
from concourse import bacc
from concourse.bass_utils import run_bass_kernel_spmd

F32 = mybir.dt.float32
BF16 = mybir.dt.bfloat16
AF = mybir.ActivationFunctionType
ALU = mybir.AluOpType

LAM = 100000.0
GMAX_BOUND = 40000.0            # safe bound on ||G|| (observed max ~2.2e4)
ALPHA = 2.0 / (2.0 * LAM + GMAX_BOUND)

N_CORES = 8
T_FULL, NQ, D = 32, 75, 1024
NW, NS = 5, 5
TPC = T_FULL // N_CORES          # tasks per core = 4
NR = TPC * NW * NS               # R rows per core = 100
NCH = D // 128                   # 8 contraction chunks
NJ = NW * NS                     # 25 (way,shot) pairs per task
NQT = TPC * NCH * NQ             # 2400 transposed-q columns

# packed bf16 constant tile [128, 776]:
#   cols   0:100  cH    = H block-diagonal (I - 1/5 11^T per 5-block)
#   cols 100:200  amask = ALPHA * blockmask
#   cols 200:300  cI    = identity
#   cols 300:400  alH   = ALPHA*LAM * H block-diagonal
#   col  400      negh  = -0.5 column (all 128 partitions)
#   row0 401:476  ones75
#   cols 476:576  alI   = ALPHA*LAM * I
#   cols 576:676  y1b   = (2 - ALPHA*LAM) * I
#   cols 676:776  twoI  = 2 * I
CPC = 776


def _host_consts():
    H5 = np.eye(NS) - np.ones((NS, NS)) / NS
    H_bd = np.kron(np.eye(TPC * NW), H5).astype(np.float32)
    blockmask = np.kron(np.eye(TPC * NW), np.ones((NS, NS))).astype(np.float32)
    eye = np.eye(NR, dtype=np.float32)
    cP = np.zeros((128, CPC), dtype=np.float32)
    cP[0:NR, 0:NR] = H_bd
    cP[0:NR, NR:2 * NR] = ALPHA * blockmask
    cP[0:NR, 2 * NR:3 * NR] = eye
    cP[0:NR, 3 * NR:4 * NR] = ALPHA * LAM * H_bd
    cP[:, 400] = -0.5
    cP[0, 401:401 + NQ] = 1.0
    cP[0:NR, 476:576] = ALPHA * LAM * eye
    cP[0:NR, 576:676] = (2.0 - ALPHA * LAM) * eye
    cP[0:NR, 676:776] = 2.0 * eye
    return cP.astype(ml_dtypes.bfloat16)


def build_nc():
    nc = bacc.Bacc("TRN2")

    r_d = nc.declare_dram_parameter("r", [NR, D], BF16, isOutput=False)
    qt_d = nc.declare_dram_parameter("qt", [128, NQT], BF16, isOutput=False)
    cP_d = nc.declare_dram_parameter("cP", [128, CPC], BF16, isOutput=False)
    out_d = nc.declare_dram_parameter("out", [NQ, TPC * NJ], F32, isOutput=True)

    with tile.TileContext(nc) as tc:
        with (
            tc.tile_pool(name="consts", bufs=1) as consts,
            tc.tile_pool(name="sb", bufs=1) as sb,
            tc.tile_pool(name="pipe", bufs=2, space="PSUM") as pipe,
            tc.tile_pool(name="gp", bufs=1, space="PSUM") as gp,
            tc.tile_pool(name="cnq", bufs=1, space="PSUM") as cnqp,
            tc.tile_pool(name="qcp", bufs=4, space="PSUM") as qcp,
        ):
            # ---- input DMAs: cP + r on the ACT HWDGE ring (its sequencer
            # is ready ~1us before Sync's); qt on the Sync ring in parallel.
            cP = consts.tile([128, CPC], BF16)
            nc.scalar.dma_start(out=cP, in_=cP_d[:])
            r_sb = sb.tile([NR, D], BF16)
            nc.scalar.dma_start(out=r_sb, in_=r_d[:])
            qt_sb = sb.tile([128, NQT], BF16)
            nc.sync.dma_start(out=qt_sb, in_=qt_d[:])

            c_H = cP[0:NR, 0:NR]
            c_amask = cP[0:NR, NR:2 * NR]
            c_I = cP[0:NR, 2 * NR:3 * NR]
            c_alH = cP[0:NR, 3 * NR:4 * NR]
            negh = cP[:, 400:401]
            ones75 = cP[0:1, 401:401 + NQ]
            c_alI = cP[0:NR, 476:576]
            c_y1b = cP[0:NR, 576:676]
            c_2I = cP[0:NR, 676:776]

            # warm the ACT spline tables (Square/Copy) before first real use
            warm = sb.tile([1, 1], BF16)
            nc.scalar.activation(warm, cP[0:1, 0:1], AF.Square)

            # ---- RcT = (H R)^T by chunks: psum[128,100] = R_chunk^T @ H ----
            rct_sb = sb.tile([128, NCH * NR], BF16)
            for p in range(2):
                rct_ps = pipe.tile([128, 4 * NR], F32, space="PSUM", tag="pp")
                for kk in range(4):
                    k = 4 * p + kk
                    nc.tensor.matmul(rct_ps[:, kk * NR:(kk + 1) * NR],
                                     lhsT=r_sb[:, k * 128:(k + 1) * 128],
                                     rhs=c_H, start=True, stop=True)
                nc.scalar.copy(rct_sb[:, p * 4 * NR:(p + 1) * 4 * NR], rct_ps)

            # ---- G = sum_k RcT_k^T RcT_k  (= H R R^T H), fp32 in PSUM ----
            g_ps = gp.tile([NR, NR], F32, space="PSUM")
            for k in range(NCH):
                rct_k = rct_sb[:, k * NR:(k + 1) * NR]
                nc.tensor.matmul(g_ps, lhsT=rct_k, rhs=rct_k,
                                 start=(k == 0), stop=(k == NCH - 1))

            # ---- q^T element squares (for ||q||^2): DVE half + ACT half ----
            sqt_sb = sb.tile([128, NQT], BF16)
            HQ = NQT // 2
            nc.vector.tensor_mul(sqt_sb[:, 0:HQ], qt_sb[:, 0:HQ],
                                 qt_sb[:, 0:HQ])
            nc.scalar.activation(sqt_sb[:, HQ:NQT], qt_sb[:, HQ:NQT],
                                 AF.Square)

            # ---- one Newton step for Y ~ (alpha K)^-1, all bf16 ----
            gm_sb = sb.tile([NR, NR], BF16)
            nc.vector.tensor_mul(gm_sb, g_ps, c_amask)       # gm = a*(mask o G)
            ka_sb = sb.tile([NR, NR], BF16)
            nc.vector.tensor_add(ka_sb, c_alI, gm_sb)        # Ka = a*lam*I + gm
            y1_sb = sb.tile([NR, NR], BF16)
            nc.vector.tensor_sub(y1_sb, c_y1b, gm_sb)        # Y1 = (2-a*lam)I - gm
            p_ps = pipe.tile([NR, NR], F32, space="PSUM", tag="pp")
            nc.tensor.matmul(p_ps, lhsT=ka_sb, rhs=y1_sb, start=True, stop=True)
            t1_ps = pipe.tile([NR, NR], F32, space="PSUM", tag="pp")
            nc.tensor.matmul(t1_ps, lhsT=y1_sb, rhs=c_alH, start=True, stop=True)
            t1_sb = sb.tile([NR, NR], BF16)
            nc.vector.tensor_copy(t1_sb, t1_ps)               # T1 = Y1*alH
            qq_sb = sb.tile([NR, NR], BF16)
            nc.vector.tensor_sub(qq_sb, c_2I, p_ps)           # Q = 2I - Ka Y1
            w2_ps = pipe.tile([NR, NR], F32, space="PSUM", tag="pp")
            nc.tensor.matmul(w2_ps, lhsT=t1_sb, rhs=qq_sb, start=True, stop=True)
            wt_sb = sb.tile([NR, NR], BF16)
            nc.vector.tensor_sub(wt_sb, c_I, w2_ps)           # W^T = I - alH Y1 Q

            # ---- C^T chunks = R_chunk^T @ W^T ; squares for ||C||^2 ----
            ct_sb = sb.tile([128, NCH * NR], BF16)
            csq_sb = sb.tile([128, NCH * NR], BF16)
            for p in range(2):
                ct_ps = pipe.tile([128, 4 * NR], F32, space="PSUM", tag="pp")
                for kk in range(4):
                    k = 4 * p + kk
                    nc.tensor.matmul(ct_ps[:, kk * NR:(kk + 1) * NR],
                                     lhsT=r_sb[:, k * 128:(k + 1) * 128],
                                     rhs=wt_sb, start=True, stop=True)
                sl = slice(p * 4 * NR, (p + 1) * 4 * NR)
                if p == 0:
                    nc.vector.tensor_copy(ct_sb[:, sl], ct_ps)
                else:
                    nc.scalar.copy(ct_sb[:, sl], ct_ps)
                nc.scalar.activation(csq_sb[:, sl], ct_ps, AF.Square)

            # ---- column reduces into one [1, 400] PSUM bank:
            #      qn2 (cols 100:400) = -0.5 sum_d qT(d,q)^2  per task
            #      cn  (cols 0:100)   = -0.5 sum_d C^T(d,j)^2
            cnq_ps = cnqp.tile([1, 4 * NR], F32, space="PSUM")
            for t in range(TPC):
                for k in range(NCH):
                    nc.tensor.matmul(
                        cnq_ps[0:1, NR + t * NQ:NR + (t + 1) * NQ],
                        lhsT=negh,
                        rhs=sqt_sb[:, (t * NCH + k) * NQ:(t * NCH + k + 1) * NQ],
                        start=(k == 0), stop=(k == NCH - 1))
            for k in range(NCH):
                nc.tensor.matmul(cnq_ps[0:1, 0:NR], lhsT=negh,
                                 rhs=csq_sb[:, k * NR:(k + 1) * NR],
                                 start=(k == 0), stop=(k == NCH - 1))
            qn2_bf = sb.tile([1, TPC * NQ], BF16)
            nc.vector.tensor_copy(qn2_bf, cnq_ps[0:1, NR:NR + TPC * NQ])
            cn_bf = sb.tile([1, NR], BF16)
            nc.vector.tensor_copy(cn_bf, cnq_ps[0:1, 0:NR])

            # ---- per task: QC accumulation + rank-1 folds + scale ----
            out_sb = sb.tile([NQ, TPC * NJ], F32)
            qc = []
            for t in range(TPC):
                qc_ps = qcp.tile([NQ, NJ], F32, space="PSUM", tag="qc",
                                 name=f"qc{t}")
                qc.append(qc_ps)
                for k in range(NCH):
                    lhs = qt_sb[:, (t * NCH + k) * NQ:(t * NCH + k + 1) * NQ]
                    rhs = ct_sb[:, k * NR + t * NJ:k * NR + t * NJ + NJ]
                    nc.tensor.matmul(qc_ps, lhsT=lhs, rhs=rhs,
                                     start=(k == 0), stop=False)
            for t in range(TPC):
                nc.tensor.matmul(qc[t], lhsT=ones75,
                                 rhs=cn_bf[0:1, t * NJ:(t + 1) * NJ],
                                 start=False, stop=False)
                nc.tensor.matmul(qc[t], lhsT=qn2_bf[0:1, t * NQ:(t + 1) * NQ],
                                 rhs=ones75[0:1, 0:NJ],
                                 start=False, stop=True)
                # logits = qc * 2/D   (qn2 and cn already folded in)
                nc.vector.tensor_scalar_mul(out_sb[:, t * NJ:(t + 1) * NJ],
                                            qc[t], 2.0 / D)
            nc.sync.dma_start(out=out_d[:], in_=out_sb)

    nc.finalize()
    return nc


_NC_CACHE = None


def _get_nc():
    global _NC_CACHE
    if _NC_CACHE is None:
        _NC_CACHE = build_nc()
    return _NC_CACHE


def make_in_maps(query, support, noise):
    query = np.asarray(query, dtype=np.float32)
    support = np.asarray(support, dtype=np.float32)
    noise = np.asarray(noise, dtype=np.float32)
    cP = _host_consts()
    in_maps = []
    for c in range(N_CORES):
        ts = slice(c * TPC, (c + 1) * TPC)
        sn = support[ts].reshape(NR, D).astype(ml_dtypes.bfloat16)
        nz = noise[:, ts].transpose(1, 0, 2, 3).reshape(NR, D) \
            .astype(ml_dtypes.bfloat16)
        r = (sn.astype(np.float32) + nz.astype(np.float32)) \
            .astype(ml_dtypes.bfloat16)
        qb = query[ts].astype(ml_dtypes.bfloat16)      # [4, 75, 1024]
        # qt[p, (t*8 + k)*75 + j] = q[t, j, k*128 + p]
        qt = np.ascontiguousarray(
            qb.transpose(2, 0, 1)                       # [1024, 4, 75]
              .reshape(NCH, 128, TPC, NQ)
              .transpose(1, 2, 0, 3)                    # [128, 4, 8, 75]
              .reshape(128, NQT))
        in_maps.append({"r": r, "qt": qt, "cP": cP})
    return in_maps


def kernel(query, support, noise, support_labels=None, n_way=None, n_shot=None,
           **_unused):
    nc = _get_nc()
    in_maps = make_in_maps(query, support, noise)
    res = run_bass_kernel_spmd(nc, in_maps, list(range(N_CORES)))
    outs = [np.asarray(r["out"]).reshape(NQ, TPC, NJ).transpose(1, 0, 2)
            for r in res.results]
    full = np.concatenate(outs, axis=0)            # (32, 75, 25)
    return full.reshape(T_FULL, NQ, NW, NS).astype(np.float32)


# revision 11
# speedup vs baseline: 1.1525x; 1.0459x over previous
"""Trainium2 Bass kernel for the ExemplarHead classification problem.

Math: per (task, way), with R the 5x1024 class reps (support+noise),
H = I - (1/5)11^T, G = H R R^T H, the SVD projection head reduces to

    C   = W R,  W = I - lam*(lam I + G)^{-1} H          (block-diag 100x100)
    logits[q, (w,s)] = (2 q.C_(w,s) - ||q||^2 - ||C_(w,s)||^2) / d

(lam I + G) has kappa <= 1.2, inverted with one scaled Newton step
(Y1 = 2I - aK, one quadratic refinement). All 20 (task,way) blocks per
core are one masked block-diagonal 100x100 problem.

All heavy matmuls run in bf16 (1 cyc/row on the PE vs 4 for fp32, and
half the LDWEIGHTS+MATMUL instruction count); accumulation stays fp32
in PSUM. q^T arrives pre-transposed from DRAM (host packs it), which
removes all PE transposes. ||q||^2 is computed from the transposed q
(element square on DVE/ACT, then a 1-column reduce matmul that hides
in the Newton-chain PE gaps); both -0.5||q||^2 and -0.5||C||^2 are
folded into the accumulating QC PSUM group as K=1 rank-1 matmuls, so
the epilogue is a single scale per task.

Sharding: data-parallel over the 32 tasks -> 4 tasks per NeuronCore x 8.
"""

import numpy as np
import ml_dtypes

import concourse.bass as bass
import concourse.mybir as mybir
import concourse.tile as tile
from concourse import bacc
from concourse.bass_utils import run_bass_kernel_spmd

F32 = mybir.dt.float32
BF16 = mybir.dt.bfloat16
AF = mybir.ActivationFunctionType
ALU = mybir.AluOpType

LAM = 100000.0
GMAX_BOUND = 40000.0            # safe bound on ||G|| (observed max ~2.2e4)
ALPHA = 2.0 / (2.0 * LAM + GMAX_BOUND)

N_CORES = 8
T_FULL, NQ, D = 32, 75, 1024
NW, NS = 5, 5
TPC = T_FULL // N_CORES          # tasks per core = 4
NR = TPC * NW * NS               # R rows per core = 100
NCH = D // 128                   # 8 contraction chunks
NJ = NW * NS                     # 25 (way,shot) pairs per task
NQT = TPC * NCH * NQ             # 2400 transposed-q columns

# packed bf16 constant tile [128, 776]:
#   cols   0:100  cH    = H block-diagonal (I - 1/5 11^T per 5-block)
#   cols 100:200  amask = ALPHA * blockmask
#   cols 200:300  cI    = identity
#   cols 300:400  alH   = ALPHA*LAM * H block-diagonal
#   col  400      negh  = -0.5 column (all 128 partitions)
#   row0 401:476  ones75
#   cols 476:576  alI   = ALPHA*LAM * I
#   cols 576:676  y1b   = (2 - ALPHA*LAM) * I
#   cols 676:776  twoI  = 2 * I
CPC = 776


def _host_consts():
    H5 = np.eye(NS) - np.ones((NS, NS)) / NS
    H_bd = np.kron(np.eye(TPC * NW), H5).astype(np.float32)
    _host_consts.H_bd = H_bd
    blockmask = np.kron(np.eye(TPC * NW), np.ones((NS, NS))).astype(np.float32)
    eye = np.eye(NR, dtype=np.float32)
    cP = np.zeros((128, CPC), dtype=np.float32)
    cP[0:NR, 0:NR] = H_bd
    cP[0:NR, NR:2 * NR] = ALPHA * blockmask
    cP[0:NR, 2 * NR:3 * NR] = eye
    cP[0:NR, 3 * NR:4 * NR] = ALPHA * LAM * H_bd
    cP[:, 400] = -0.5
    cP[0, 401:401 + NQ] = 1.0
    cP[0:NR, 476:576] = ALPHA * LAM * eye
    cP[0:NR, 576:676] = (2.0 - ALPHA * LAM) * eye
    cP[0:NR, 676:776] = 2.0 * eye
    return cP.astype(ml_dtypes.bfloat16)


def build_nc():
    nc = bacc.Bacc("TRN2")

    r_d = nc.declare_dram_parameter("r", [NR, D], BF16, isOutput=False)
    cH_d = nc.declare_dram_parameter("cH", [NR, NR], BF16, isOutput=False)
    qt_d = nc.declare_dram_parameter("qt", [128, NQT], BF16, isOutput=False)
    cP_d = nc.declare_dram_parameter("cP", [128, CPC], BF16, isOutput=False)
    out_d = nc.declare_dram_parameter("out", [NQ, TPC * NJ], F32, isOutput=True)

    with tile.TileContext(nc) as tc:
        with (
            tc.tile_pool(name="consts", bufs=1) as consts,
            tc.tile_pool(name="sb", bufs=1) as sb,
            tc.tile_pool(name="pipe", bufs=2, space="PSUM") as pipe,
            tc.tile_pool(name="gp", bufs=1, space="PSUM") as gp,
            tc.tile_pool(name="cnq", bufs=1, space="PSUM") as cnqp,
            tc.tile_pool(name="qcp", bufs=4, space="PSUM") as qcp,
        ):
            # ---- input DMAs: cP + r on the ACT HWDGE ring (its sequencer
            # is ready ~1us before Sync's); qt on the Sync ring in parallel.
            r_sb = sb.tile([NR, D], BF16)
            nc.scalar.dma_start(out=r_sb, in_=r_d[:])
            cP = consts.tile([128, CPC], BF16)
            nc.scalar.dma_start(out=cP, in_=cP_d[:])
            cH_sb = consts.tile([NR, NR], BF16)
            nc.sync.dma_start(out=cH_sb, in_=cH_d[:])
            qt_sb = sb.tile([128, NQT], BF16)
            nc.sync.dma_start(out=qt_sb, in_=qt_d[:])

            c_amask = cP[0:NR, NR:2 * NR]
            c_I = cP[0:NR, 2 * NR:3 * NR]
            c_alH = cP[0:NR, 3 * NR:4 * NR]
            negh = cP[:, 400:401]
            ones75 = cP[0:1, 401:401 + NQ]
            c_alI = cP[0:NR, 476:576]
            c_y1b = cP[0:NR, 576:676]
            c_2I = cP[0:NR, 676:776]

            # warm the ACT spline tables (Square/Copy) before first real use
            warm = sb.tile([1, 1], BF16)
            nc.scalar.activation(warm, cP[0:1, 0:1], AF.Square)

            # ---- RcT = (H R)^T by chunks: psum[128,100] = R_chunk^T @ H ----
            rct_sb = sb.tile([128, NCH * NR], BF16)
            for p in range(2):
                rct_ps = pipe.tile([128, 4 * NR], F32, space="PSUM", tag="pp")
                for kk in range(4):
                    k = 4 * p + kk
                    nc.tensor.matmul(rct_ps[:, kk * NR:(kk + 1) * NR],
                                     lhsT=r_sb[:, k * 128:(k + 1) * 128],
                                     rhs=cH_sb[:], start=True, stop=True)
                if p == 0:
                    nc.vector.tensor_copy(rct_sb[:, 0:4 * NR], rct_ps)
                else:
                    nc.scalar.copy(rct_sb[:, 4 * NR:8 * NR], rct_ps)

            # ---- G = sum_k RcT_k^T RcT_k  (= H R R^T H), fp32 in PSUM ----
            g_ps = gp.tile([NR, NR], F32, space="PSUM")
            for k in range(NCH):
                rct_k = rct_sb[:, k * NR:(k + 1) * NR]
                nc.tensor.matmul(g_ps, lhsT=rct_k, rhs=rct_k,
                                 start=(k == 0), stop=(k == NCH - 1))

            # ---- q^T element squares (for ||q||^2): DVE half + ACT half ----
            sqt_sb = sb.tile([128, NQT], BF16)
            HQ = NQT // 2
            nc.vector.tensor_mul(sqt_sb[:, 0:HQ], qt_sb[:, 0:HQ],
                                 qt_sb[:, 0:HQ])
            nc.scalar.activation(sqt_sb[:, HQ:NQT], qt_sb[:, HQ:NQT],
                                 AF.Square)

            cnq_ps = cnqp.tile([1, 4 * NR], F32, space="PSUM")

            def qnred(t):
                # qn2[t] = -0.5 sum_d qT(d,q)^2, accumulated on the PE
                for k in range(NCH):
                    nc.tensor.matmul(
                        cnq_ps[0:1, NR + t * NQ:NR + (t + 1) * NQ],
                        lhsT=negh,
                        rhs=sqt_sb[:, (t * NCH + k) * NQ:(t * NCH + k + 1) * NQ],
                        start=(k == 0), stop=(k == NCH - 1))

            # ---- one Newton step for Y ~ (alpha K)^-1, all bf16;
            #      qn2 reduce blocks fill the PE gaps in the chain ----
            gm_sb = sb.tile([NR, NR], BF16)
            nc.vector.tensor_mul(gm_sb, g_ps, c_amask)       # gm = a*(mask o G)
            ka_sb = sb.tile([NR, NR], BF16)
            nc.vector.tensor_add(ka_sb, c_alI, gm_sb)        # Ka = a*lam*I + gm
            y1_sb = sb.tile([NR, NR], BF16)
            nc.vector.tensor_sub(y1_sb, c_y1b, gm_sb)        # Y1 = (2-a*lam)I - gm
            qnred(0)
            p_ps = pipe.tile([NR, NR], F32, space="PSUM", tag="pp")
            nc.tensor.matmul(p_ps, lhsT=ka_sb, rhs=y1_sb, start=True, stop=True)
            t1_ps = pipe.tile([NR, NR], F32, space="PSUM", tag="pp")
            nc.tensor.matmul(t1_ps, lhsT=y1_sb, rhs=c_alH, start=True, stop=True)
            t1_sb = sb.tile([NR, NR], BF16)
            nc.scalar.copy(t1_sb, t1_ps)                      # T1 = Y1*alH
            qq_sb = sb.tile([NR, NR], BF16)
            nc.vector.tensor_sub(qq_sb, c_2I, p_ps)           # Q = 2I - Ka Y1
            qnred(1)
            w2_ps = pipe.tile([NR, NR], F32, space="PSUM", tag="pp")
            nc.tensor.matmul(w2_ps, lhsT=t1_sb, rhs=qq_sb, start=True, stop=True)
            qnred(2)
            wt_sb = sb.tile([NR, NR], BF16)
            nc.vector.tensor_sub(wt_sb, c_I, w2_ps)           # W^T = I - alH Y1 Q

            # ---- C^T chunks = R_chunk^T @ W^T ; squares for ||C||^2 ----
            ct_sb = sb.tile([128, NCH * NR], BF16)
            csq_sb = sb.tile([128, NCH * NR], BF16)
            for p in range(2):
                ct_ps = pipe.tile([128, 4 * NR], F32, space="PSUM", tag="pp")
                for kk in range(4):
                    k = 4 * p + kk
                    nc.tensor.matmul(ct_ps[:, kk * NR:(kk + 1) * NR],
                                     lhsT=r_sb[:, k * 128:(k + 1) * 128],
                                     rhs=wt_sb, start=True, stop=True)
                sl = slice(p * 4 * NR, (p + 1) * 4 * NR)
                nc.vector.tensor_copy(ct_sb[:, sl], ct_ps)
                nc.scalar.activation(csq_sb[:, sl], ct_ps, AF.Square)
            qnred(3)

            # ---- cn = -0.5 sum_d C^T(d,j)^2 into cols 0:100 of the bank ----
            for k in range(NCH):
                nc.tensor.matmul(cnq_ps[0:1, 0:NR], lhsT=negh,
                                 rhs=csq_sb[:, k * NR:(k + 1) * NR],
                                 start=(k == 0), stop=(k == NCH - 1))
            qn2_bf = sb.tile([1, TPC * NQ], BF16)
            nc.vector.tensor_copy(qn2_bf, cnq_ps[0:1, NR:NR + TPC * NQ])
            cn_bf = sb.tile([1, NR], BF16)
            nc.vector.tensor_copy(cn_bf, cnq_ps[0:1, 0:NR])

            # ---- per task: QC accumulation + rank-1 folds + scale ----
            out_sb = sb.tile([NQ, TPC * NJ], F32)
            for t in range(TPC):
                qc_ps = qcp.tile([NQ, NJ], F32, space="PSUM", tag="qc",
                                 name=f"qc{t}")
                for k in range(NCH):
                    lhs = qt_sb[:, (t * NCH + k) * NQ:(t * NCH + k + 1) * NQ]
                    rhs = ct_sb[:, k * NR + t * NJ:k * NR + t * NJ + NJ]
                    nc.tensor.matmul(qc_ps, lhsT=lhs, rhs=rhs,
                                     start=(k == 0), stop=False)
                nc.tensor.matmul(qc_ps, lhsT=ones75,
                                 rhs=cn_bf[0:1, t * NJ:(t + 1) * NJ],
                                 start=False, stop=False)
                nc.tensor.matmul(qc_ps, lhsT=qn2_bf[0:1, t * NQ:(t + 1) * NQ],
                                 rhs=ones75[0:1, 0:NJ],
                                 start=False, stop=True)
                # logits = qc * 2/D   (qn2 and cn already folded in)
                nc.vector.tensor_scalar_mul(out_sb[:, t * NJ:(t + 1) * NJ],
                                            qc_ps, 2.0 / D)
            nc.sync.dma_start(out=out_d[:], in_=out_sb)

    nc.finalize()
    return nc


_NC_CACHE = None


def _get_nc():
    global _NC_CACHE
    if _NC_CACHE is None:
        _NC_CACHE = build_nc()
    return _NC_CACHE


def make_in_maps(query, support, noise):
    query = np.asarray(query, dtype=np.float32)
    support = np.asarray(support, dtype=np.float32)
    noise = np.asarray(noise, dtype=np.float32)
    cP = _host_consts()
    in_maps = []
    for c in range(N_CORES):
        ts = slice(c * TPC, (c + 1) * TPC)
        sn = support[ts].reshape(NR, D).astype(ml_dtypes.bfloat16)
        nz = noise[:, ts].transpose(1, 0, 2, 3).reshape(NR, D) \
            .astype(ml_dtypes.bfloat16)
        r = (sn.astype(np.float32) + nz.astype(np.float32)) \
            .astype(ml_dtypes.bfloat16)
        qb = query[ts].astype(ml_dtypes.bfloat16)      # [4, 75, 1024]
        # qt[p, (t*8 + k)*75 + j] = q[t, j, k*128 + p]
        qt = np.ascontiguousarray(
            qb.transpose(2, 0, 1)                       # [1024, 4, 75]
              .reshape(NCH, 128, TPC, NQ)
              .transpose(1, 2, 0, 3)                    # [128, 4, 8, 75]
              .reshape(128, NQT))
        in_maps.append({"r": r, "qt": qt, "cP": cP,
                        "cH": _host_consts.H_bd.astype(ml_dtypes.bfloat16)})
    return in_maps


def kernel(query, support, noise, support_labels=None, n_way=None, n_shot=None,
           **_unused):
    nc = _get_nc()
    in_maps = make_in_maps(query, support, noise)
    res = run_bass_kernel_spmd(nc, in_maps, list(range(N_CORES)))
    outs = [np.asarray(r["out"]).reshape(NQ, TPC, NJ).transpose(1, 0, 2)
            for r in res.results]
    full = np.concatenate(outs, axis=0)            # (32, 75, 25)
    return full.reshape(T_FULL, NQ, NW, NS).astype(np.float32)
